# revision 39
# baseline (speedup 1.0000x reference)
"""MLA prefill attention kernel for 8 TRN2 NeuronCores (v2).

Sharding: phase 1 is data-parallel over rows (B*S = 4096 rows, 512/core):
x -> q_lora -> rmsnorm -> q_b (all heads) -> rope, and
x -> kv_lora -> rmsnorm / k_pe rope.  The per-row latents are then
exchanged: AllToAll moves Q^T from row-sharded to head-sharded layout,
AllGather replicates the (small) compressed kv latents.  Phase 2 is
tensor-parallel over heads (2 heads/core): expand K/V from the latents,
causal flash-style attention in score-transposed layout, then each core
computes a partial x @ wo^T for its heads' slice; the host sums the 8
fp16 partials.

v2 changes vs baseline:
 - rope projections emit one fused [x0;x1] psum tile; the half-swap for
   the rotation runs as a SBUF->SBUF DMA, and the rotation itself is 3
   full-width DVE ops against [c;c]/[-s;s] tables (halves the pe matmul
   count in 1a/1c).
 - 1c computes nE/pe/nO in a single pass per shard (3 accumulating
   psum tiles) instead of two passes.
 - attention processes both heads per kv-tile; the K=64 rope-score
   matmuls of the two heads run row-packed (tile_position (0,0)/(64,0))
   and the softmax-sum matmuls run col-packed ((0,0)/(0,32)), each pair
   costing one matmul slot.
 - causal masking of diagonal tiles moved off the PE: exp runs
   unmasked, then gpsimd affine_select zeroes the upper triangle.
 - wo drain: per-head softmax normalizers are broadcast via a rank-1
   matmul, oT is normalized once per head, and both heads accumulate
   into a single psum group with a single fp16 drain copy.
 - partial outputs are fp16 (halves drain + output DMA cost).
"""

import numpy as np

import concourse.bass as bass
import concourse.mybir as mybir
import concourse.tile as tile
from concourse import bacc
from concourse.bass_utils import run_bass_kernel_spmd

# ---- problem constants --------------------------------------------------
NCORE = 8
B, S, DIM = 2, 2048, 2048
H = 16
QL = 1536           # q lora rank
KVL = 512           # kv lora rank
NOPE, ROPE = 128, 64
QKD = NOPE + ROPE   # 192
VD = 128
SCALE = QKD ** -0.5
EPS = float(np.finfo(np.float32).eps)
ROWS = B * S        # 4096
R = ROWS // NCORE   # 512 rows per core
HC = H // NCORE     # 2 heads per core
NW = S // 512       # 4 query windows of 512 per batch

F32 = mybir.dt.float32
F16 = mybir.dt.float16
MM_DT = mybir.dt.bfloat16
import ml_dtypes
NP_MM_DT = ml_dtypes.bfloat16

_compiled = {}


def _build_nc():
    nc = bacc.Bacc("TRN2", target_bir_lowering=False, debug=False,
                   num_devices=NCORE)

    dram_in = lambda name, shape, dt=MM_DT: nc.dram_tensor(
        name, shape, dt, kind="ExternalInput").ap()

    # projection operands are host-packed partition-major ([128, ...]) so
    # every DMA moves multi-KB contiguous runs per partition (the naive
    # [dim, R] layouts produce 1KB descriptors at ~half DMA bandwidth)
    xP = dram_in("xP", [128, 4 * 4 * R])            # x^T, 4 chunks/tile
    wqaP = dram_in("wqaP", [128, 4 * 4 * QL])       # wq_a^T packed
    wkvaP = dram_in("wkvaP", [128, 4 * 4 * (KVL + ROPE)])
    wqbP = dram_in("wqbP", [128, 8 * 12 * 384])     # (g, kq, kr, c) packed
    wkbT = dram_in("wkbT", [KVL, HC * NOPE])        # my heads' k expand
    wvbT = dram_in("wvbT", [KVL, HC * VD])          # my heads' v expand
    woT = dram_in("woT", [HC * VD, DIM])            # my heads' wo slice^T
    cosA = dram_in("cosA", [64, R])     # [c32; c32]
    sinApm = dram_in("sinApm", [64, R])  # [-s32; s32]
    cosC = dram_in("cosC", [128, R])    # [c32 x4]
    sinCpm = dram_in("sinCpm", [128, R])  # [-s32;-s32; s32;s32]
    out = nc.dram_tensor("out", [ROWS, DIM], F16, kind="ExternalOutput").ap()

    QD = H * QKD        # 3072 rows of Q^T (permuted/grouped)
    KVD = KVL + ROPE    # 576

    from contextlib import ExitStack
    with tile.TileContext(nc) as tc, ExitStack() as stk:
        dramp = stk.enter_context(tc.tile_pool(name="dram", bufs=1,
                                               space="DRAM"))
        constp = stk.enter_context(tc.tile_pool(name="const", bufs=1))
        persist = stk.enter_context(tc.tile_pool(name="persist", bufs=1))
        workp = stk.enter_context(tc.tile_pool(name="work", bufs=3))
        # stays open through phase 2 (ph2/ps_mm open above it mid-build,
        # so a mid-build close would violate pool LIFO order)
        p1qa = stk.enter_context(tc.tile_pool(name="p1_qa", bufs=1))
        ps1ab_stk = ExitStack()
        ps1 = ps1ab_stk.enter_context(tc.tile_pool(name="ps1ab", bufs=1,
                                                   space="PSUM"))
        p1x_stk = ExitStack()
        p1x = p1x_stk.enter_context(tc.tile_pool(name="p1_x", bufs=1))
        p1kv_stk = ExitStack()
        p1kv = p1kv_stk.enter_context(tc.tile_pool(name="p1_kv", bufs=1))
        if True:

            # ---------------- constants ----------------
            ones_f32 = constp.tile([128, 1], F32, name="ones_f32",
                                   tag="ones_f32")
            nc.gpsimd.memset(ones_f32, 1.0)
            ones_row_f32 = constp.tile([1, 128], F32, name="ones_row_f32",
                                       tag="ones_row_f32")
            nc.gpsimd.memset(ones_row_f32, 1.0)
            ones_col = constp.tile([128, 1], MM_DT, name="ones_col",
                                   tag="ones_col")
            nc.vector.tensor_copy(ones_col[:], ones_f32[:])
            ones_row = constp.tile([1, 128], MM_DT, name="ones_row",
                                   tag="ones_row")
            nc.vector.tensor_copy(ones_row[:], ones_row_f32[:])
            eps1 = constp.tile([1, 1], F32, name="eps1", tag="eps1")
            nc.gpsimd.memset(eps1, EPS)
            cosA_sb = constp.tile([64, R], MM_DT, name="cosA_sb",
                                  tag="cosA_sb")
            sinA_sb = constp.tile([64, R], MM_DT, name="sinA_sb",
                                  tag="sinA_sb")
            cosC_sb = constp.tile([128, R], MM_DT, name="cosC_sb",
                                  tag="cosC_sb")
            sinC_sb = constp.tile([128, R], MM_DT, name="sinC_sb",
                                  tag="sinC_sb")
            nc.sync.dma_start(out=cosA_sb[:], in_=cosA[:])
            nc.sync.dma_start(out=sinA_sb[:], in_=sinApm[:])
            nc.sync.dma_start(out=cosC_sb[:], in_=cosC[:])
            nc.sync.dma_start(out=sinC_sb[:], in_=sinCpm[:])

            # x^T resident: 4 packed tiles of 4 chunks each
            x4 = []
            for q in range(4):
                t = p1x.tile([128, 4 * R], MM_DT, name=f"x4_{q}",
                             tag=f"x4_{q}")
                nc.sync.dma_start(out=t[:],
                                  in_=xP[:, q * 4 * R:(q + 1) * 4 * R])
                x4.append(t)

            def x_chunk(k):
                return x4[k // 4][:, (k % 4) * R:(k % 4 + 1) * R]

            # collective buffers
            kvag_in = dramp.tile([KVD, R], MM_DT, name="kvag_in",
                                 tag="kvag_in")
            kvag_out = dramp.tile([NCORE * KVD, R], MM_DT, name="kvag_out",
                                  tag="kvag_out", addr_space="Shared")
            qa2a_in = dramp.tile([QD, R], MM_DT, name="qa2a_in",
                                 tag="qa2a_in")
            qa2a_out = dramp.tile([QD, R], MM_DT, name="qa2a_out",
                                  tag="qa2a_out")

            # ---------------- phase 1a: kv latents (feeds AllGather) -----
            kv_dt = []     # kvnT tiles [128, R] per kvl chunk
            ssq_kv = ps1.tile([1, R], F32, name="ssq_kv", tag="ssq_small")
            ps_pe1a = ps1.tile([64, R], F32, name="ps_pe1a", tag="pe_x")
            wkva4 = []
            for q in range(4):
                wt = p1kv.tile([128, 4 * KVD], MM_DT, name=f"wkva4_{q}",
                               tag=f"wkva4_{q}")
                nc.sync.dma_start(
                    out=wt[:], in_=wkvaP[:, q * 4 * KVD:(q + 1) * 4 * KVD])
                wkva4.append(wt)

            def wkva_sl(k, c0, c1):
                base = (k % 4) * KVD
                return wkva4[k // 4][:, base + c0:base + c1]

            for k in range(DIM // 128):
                nc.tensor.matmul(ps_pe1a[:], wkva_sl(k, KVL, KVD),
                                 x_chunk(k), start=(k == 0), stop=(k == 15))
            sq_kv = []
            for blk in range(2):
                ps_kv = [ps1.tile([128, R], F32, name=f"ps_kv{d}", tag="acc",
                                  bufs=4) for d in range(2)]
                for k in range(DIM // 128):
                    for d in range(2):
                        dd = blk * 2 + d
                        nc.tensor.matmul(ps_kv[d][:],
                                         wkva_sl(k, dd * 128, (dd + 1) * 128),
                                         x_chunk(k),
                                         start=(k == 0), stop=(k == 15))
                for d in range(2):
                    dd = blk * 2 + d
                    t = p1kv.tile([128, R], MM_DT, name=f"kvnT{dd}",
                                  tag=f"kvnT{dd}")
                    nc.scalar.activation(t[:], ps_kv[d][:],
                                         mybir.ActivationFunctionType.Copy)
                    sq = p1qa.tile([128, R], MM_DT, name="sq_kv",
                                   tag=f"sq_kv{dd}")
                    nc.vector.tensor_mul(sq[:], t[:], t[:])
                    sq_kv.append(sq)
                    kv_dt.append(t)
            # ssq matmuls issued after the projection loop so they don't
            # stall the in-order PE queue mid-stream
            for dd in range(4):
                nc.tensor.matmul(ssq_kv[:], ones_col[:], sq_kv[dd][:],
                                 start=(dd == 0), stop=(dd == 3))
            # rsqrt + broadcast along partitions via rank-1 matmul
            rs_kv = workp.tile([1, R], F32, name="rs_kv", tag="rs_small",
                               bufs=2)
            nc.scalar.activation(rs_kv[:], ssq_kv[:],
                                 mybir.ActivationFunctionType.Sqrt,
                                 bias=eps1[:], scale=1.0 / KVL)
            ri_kv = workp.tile([1, R], F32, name="ri_kv", tag="ri_small",
                               bufs=2)
            nc.vector.reciprocal_approx_fast(ri_kv[:], rs_kv[:])
            rib_kv = workp.tile([1, R], MM_DT, name="rib_kv",
                                tag="rib_small", bufs=2)
            nc.vector.tensor_copy(rib_kv[:], ri_kv[:])
            bc_ps = ps1.tile([128, R], F32, name="bc_kv", tag="bc_ps")
            nc.tensor.matmul(bc_ps[:], ones_row[:], rib_kv[:],
                             start=True, stop=True)
            bc_sb = p1qa.tile([128, R], MM_DT, name="bc_kv_sb", tag="bc",
                              bufs=2)
            nc.scalar.activation(bc_sb[:], bc_ps[:],
                                 mybir.ActivationFunctionType.Copy)
            for d in range(4):
                nc.vector.tensor_mul(kv_dt[d][:], kv_dt[d][:], bc_sb[:])
                nc.sync.dma_start(out=kvag_in[d * 128:(d + 1) * 128, :],
                                  in_=kv_dt[d][:])
            # k_pe rope: [x0;x1] -> y = x*[c;c] + swap(x)*[-s;s]
            kpe_all = p1kv.tile([64, R], MM_DT, name="kpe_all", tag="kpe_all")
            nc.scalar.activation(kpe_all[:], ps_pe1a[:],
                                 mybir.ActivationFunctionType.Copy)
            kswap = p1kv.tile([64, R], MM_DT, name="kswap", tag="kswap")
            nc.sync.dma_start(out=kswap[0:32, :], in_=kpe_all[32:64, :])
            nc.sync.dma_start(out=kswap[32:64, :], in_=kpe_all[0:32, :])
            ktmp = p1kv.tile([64, R], MM_DT, name="ktmp", tag="ktmp")
            nc.vector.tensor_mul(ktmp[:], kswap[:], sinA_sb[:])
            ky = p1kv.tile([64, R], MM_DT, name="ky", tag="ky")
            nc.vector.tensor_mul(ky[:], kpe_all[:], cosA_sb[:])
            nc.vector.tensor_add(ky[:], ky[:], ktmp[:])
            nc.sync.dma_start(out=kvag_in[KVL:KVD, :], in_=ky[:])
            nc.gpsimd.collective_compute(
                "AllGather", mybir.AluOpType.bypass,
                replica_groups=[list(range(NCORE))],
                ins=[kvag_in.opt()], outs=[kvag_out.opt()])
            p1kv_stk.close()

            # ---------------- phase 1b: q latents ------------------------
            qa_dt = []
            sq_q = []
            ssq_q = ps1.tile([1, R], F32, name="ssq_q", tag="ssq_small")
            wqa4 = []
            for q in range(4):
                wt = p1x.tile([128, 4 * QL], MM_DT, name=f"wqa4_{q}",
                              tag=f"wqa4_{q}")
                nc.sync.dma_start(
                    out=wt[:], in_=wqaP[:, q * 4 * QL:(q + 1) * 4 * QL])
                wqa4.append(wt)

            def wqa_sl(k, c0, c1):
                base = (k % 4) * QL
                return wqa4[k // 4][:, base + c0:base + c1]

            for cb in range(3):         # 512-col weight block
                for sub in range(2):    # 2 dtiles at a time
                    ps_q = [ps1.tile([128, R], F32, name=f"ps_q{d}",
                            tag="acc", bufs=4) for d in range(2)]
                    for k in range(DIM // 128):
                        for d in range(2):
                            off = cb * 512 + sub * 256 + d * 128
                            nc.tensor.matmul(ps_q[d][:],
                                             wqa_sl(k, off, off + 128),
                                             x_chunk(k),
                                             start=(k == 0), stop=(k == 15))
                    for d in range(2):
                        dt_i = cb * 4 + sub * 2 + d
                        t = p1qa.tile([128, R], MM_DT, name=f"qaT{dt_i}",
                                      tag=f"qaT{dt_i}")
                        nc.scalar.activation(
                            t[:], ps_q[d][:],
                            mybir.ActivationFunctionType.Copy)
                        sq = p1qa.tile([128, R], MM_DT, name="sq_q",
                                       tag=f"sq_q{dt_i}")
                        nc.vector.tensor_mul(sq[:], t[:], t[:])
                        sq_q.append(sq)
                        qa_dt.append(t)
            for dt_i in range(12):
                nc.tensor.matmul(ssq_q[:], ones_col[:], sq_q[dt_i][:],
                                 start=(dt_i == 0), stop=(dt_i == 11))
            rs_q = workp.tile([1, R], F32, name="rs_q", tag="rs_small",
                              bufs=2)
            nc.scalar.activation(rs_q[:], ssq_q[:],
                                 mybir.ActivationFunctionType.Sqrt,
                                 bias=eps1[:], scale=1.0 / QL)
            ri_q = workp.tile([1, R], F32, name="ri_q", tag="ri_small",
                              bufs=2)
            nc.vector.reciprocal_approx_fast(ri_q[:], rs_q[:])
            rib_q = workp.tile([1, R], MM_DT, name="rib_q",
                               tag="rib_small", bufs=2)
            nc.vector.tensor_copy(rib_q[:], ri_q[:])
            bcq_ps = ps1.tile([128, R], F32, name="bc_q", tag="bc_ps")
            nc.tensor.matmul(bcq_ps[:], ones_row[:], rib_q[:],
                             start=True, stop=True)
            # the per-row rmsnorm scale commutes through q_b (it's a
            # per-token scalar), so it's applied at the 1c drains below
            # instead of to qa_dt — removes the serial 1b->1c boundary.
            bcq_sb = p1qa.tile([128, R], MM_DT, name="bc_q_sb", tag="bc",
                               bufs=2)
            nc.scalar.activation(bcq_sb[:], bcq_ps[:],
                                 mybir.ActivationFunctionType.Copy)

            p1x_stk.close()
            ps1ab_stk.close()
            # score/expansion psum pool opens before 1c so the K/V
            # expansion matmuls can fill 1c stalls and the AllToAll wait;
            # ph2 likewise so the kvag_out loads prefetch during 1c.
            ps_mm = stk.enter_context(tc.tile_pool(name="ps_mm", bufs=3,
                                                   space="PSUM"))
            ph2 = stk.enter_context(tc.tile_pool(name="ph2", bufs=1))
            p1w_stk = ExitStack()
            p1w = p1w_stk.enter_context(tc.tile_pool(name="p1_w", bufs=1))
            ps1c_stk = ExitStack()
            ps1c = ps1c_stk.enter_context(tc.tile_pool(name="ps1c", bufs=1,
                                                       space="PSUM"))

            # ---------------- phase 1c: q_b + rope -> AllToAll ------------
            # single pass per shard: nE / pe / nO accumulate in parallel
            for g in range(NCORE):
                wt4 = []
                for kq in range(3):
                    wt = p1w.tile([128, 1536], MM_DT, name="wqb4",
                                  tag="wqb4", bufs=8)
                    nc.sync.dma_start(
                        out=wt[:],
                        in_=wqbP[:, g * 4608 + kq * 1536:
                                 g * 4608 + (kq + 1) * 1536])
                    wt4.append(wt)

                def wqb_sl(k, c0, c1):
                    base = (k % 4) * 384
                    return wt4[k // 4][:, base + c0:base + c1]

                ps_nE = ps1c.tile([128, R], F32, name="ps_nE", tag="accE",
                                  bufs=2)
                ps_pe = ps1c.tile([128, R], F32, name="ps_qpe", tag="accP",
                                  bufs=2)
                ps_nO = ps1c.tile([128, R], F32, name="ps_nO", tag="accO",
                                  bufs=1)
                for k in range(QL // 128):
                    nc.tensor.matmul(ps_nE[:], wqb_sl(k, 0, 128),
                                     qa_dt[k][:],
                                     start=(k == 0), stop=(k == 11))
                    nc.tensor.matmul(ps_pe[:], wqb_sl(k, 128, 256),
                                     qa_dt[k][:],
                                     start=(k == 0), stop=(k == 11))
                    nc.tensor.matmul(ps_nO[:], wqb_sl(k, 256, 384),
                                     qa_dt[k][:],
                                     start=(k == 0), stop=(k == 11))
                stE = p1qa.tile([128, R], MM_DT, name="qoutE", tag="qout",
                                bufs=4)
                nc.vector.tensor_mul(stE[:], ps_nE[:], bcq_sb[:])
                nc.sync.dma_start(
                    out=qa2a_in[g * 384:g * 384 + 128, :], in_=stE[:])
                stO = p1qa.tile([128, R], MM_DT, name="qoutO", tag="qout",
                                bufs=4)
                nc.vector.tensor_mul(stO[:], ps_nO[:], bcq_sb[:])
                nc.sync.dma_start(
                    out=qa2a_in[g * 384 + 256:g * 384 + 384, :], in_=stO[:])
                # rope: y = x*[c;c;c;c] + swap64(x)*[-s;-s;s;s]
                # (the rmsnorm scale rides along through the rotation)
                qpe_all = p1qa.tile([128, R], MM_DT, name="qpe_all",
                                    tag="qpe_all", bufs=2)
                nc.vector.tensor_mul(qpe_all[:], ps_pe[:], bcq_sb[:])
                qswap = p1qa.tile([128, R], MM_DT, name="qswap", tag="qswap",
                                  bufs=2)
                nc.sync.dma_start(out=qswap[0:64, :], in_=qpe_all[64:128, :])
                nc.sync.dma_start(out=qswap[64:128, :], in_=qpe_all[0:64, :])
                qtmp = p1qa.tile([128, R], MM_DT, name="qtmp", tag="qtmp",
                                 bufs=2)
                nc.vector.tensor_mul(qtmp[:], qswap[:], sinC_sb[:])
                qy = p1qa.tile([128, R], MM_DT, name="qy", tag="qy", bufs=2)
                nc.vector.tensor_mul(qy[:], qpe_all[:], cosC_sb[:])
                nc.vector.tensor_add(qy[:], qy[:], qtmp[:])
                nc.sync.dma_start(
                    out=qa2a_in[g * 384 + 128:g * 384 + 256, :], in_=qy[:])
            nc.gpsimd.collective_compute(
                "AllToAll", mybir.AluOpType.bypass,
                replica_groups=[list(range(NCORE))],
                ins=[qa2a_in.opt()], outs=[qa2a_out.opt()])
            ps1c_stk.close()
            p1w_stk.close()
            ps_o = stk.enter_context(tc.tile_pool(name="ps_o", bufs=1,
                                                  space="PSUM"))
            ps_wo = stk.enter_context(tc.tile_pool(name="ps_wo", bufs=2,
                                                   space="PSUM"))
            ps_sm = stk.enter_context(tc.tile_pool(name="ps_sm", bufs=1,
                                                   space="PSUM"))

            # phase-2 weights: traced after the AllToAll so the scheduler
            # models the expansion matmuls as post-1c work (an earlier
            # trace position lets them land mid-1c in the static PE order
            # and head-of-line-block it behind the AllGather)
            wkb_sb = []
            wvb_sb = []
            for m in range(4):
                t = persist.tile([128, HC * NOPE], MM_DT, name=f"wkb{m}",
                                 tag=f"wkb{m}")
                nc.sync.dma_start(out=t[:], in_=wkbT[m * 128:(m + 1) * 128, :])
                wkb_sb.append(t)
                t2 = persist.tile([128, HC * VD], MM_DT, name=f"wvb{m}",
                                  tag=f"wvb{m}")
                nc.sync.dma_start(out=t2[:],
                                  in_=wvbT[m * 128:(m + 1) * 128, :])
                wvb_sb.append(t2)
            wo_sb = []
            for hh in range(HC):
                t = persist.tile([128, DIM], MM_DT, name=f"wo{hh}",
                                 tag=f"wo{hh}")
                nc.sync.dma_start(out=t[:],
                                  in_=woT[hh * 128:(hh + 1) * 128, :])
                wo_sb.append(t)

            # ---------------- phase 2: expansion (per batch) -------------
            # batch 0 right after the AllToAll; batch 1 is traced after
            # batch-0 attention so its matmuls are the lowest-priority
            # filler for the AllToAll wait and attention stalls.
            kT = {}      # (b, hh) -> [128, S]
            v_sb = {}    # (b, rr) -> [128, HC*VD]
            kpe_dup = {}  # b -> [128, S] (kpe duplicated in both halves)

            def expand_batch(b):
                kvg = []
                for jj in range(4):
                    j = NW * b + jj
                    row0 = j * KVD
                    tiles_m = []
                    for m in range(4):
                        t = ph2.tile([128, R], MM_DT, name="kvg",
                                     tag=f"kvg{b}_{jj}_{m}", bufs=1)
                        nc.sync.dma_start(
                            out=t[:],
                            in_=kvag_out[row0 + m * 128:row0 + (m + 1) * 128,
                                         :])
                        tiles_m.append(t)
                    kvg.append(tiles_m)
                kp = ph2.tile([128, S], MM_DT, name="kpe_dup",
                              tag=f"kpe_dup{b}", bufs=1)
                for jj in range(4):
                    j = NW * b + jj
                    row0 = j * KVD
                    nc.sync.dma_start(
                        out=kp[0:64, jj * R:(jj + 1) * R],
                        in_=kvag_out[row0 + KVL:row0 + KVD, :])
                    nc.sync.dma_start(
                        out=kp[64:128, jj * R:(jj + 1) * R],
                        in_=kvag_out[row0 + KVL:row0 + KVD, :])
                kpe_dup[b] = kp

                for hh in range(HC):
                    t = ph2.tile([128, S], MM_DT, name=f"kT{hh}",
                                 tag=f"kT{b}_{hh}", bufs=1)
                    for jj in range(4):
                        ps = ps_mm.tile([128, R], F32, name="ps_kT",
                                        tag="mm")
                        for m in range(4):
                            nc.tensor.matmul(
                                ps[:],
                                wkb_sb[m][:, hh * NOPE:(hh + 1) * NOPE],
                                kvg[jj][m][:],
                                start=(m == 0), stop=(m == 3))
                        if jj % 2 == 0:
                            nc.vector.tensor_copy(
                                t[:, jj * R:(jj + 1) * R], ps[:])
                        else:
                            nc.scalar.activation(
                                t[:, jj * R:(jj + 1) * R], ps[:],
                                mybir.ActivationFunctionType.Copy)
                    kT[(b, hh)] = t

                for rr in range(S // 128):
                    jj, sl = rr // 4, rr % 4
                    ps = ps_mm.tile([128, HC * VD], F32, name="ps_v",
                                    tag="mm")
                    for m in range(4):
                        nc.tensor.matmul(
                            ps[:],
                            kvg[jj][m][:, sl * 128:(sl + 1) * 128],
                            wvb_sb[m][:],
                            start=(m == 0), stop=(m == 3))
                    t = ph2.tile([128, HC * VD], MM_DT, name="v_sb",
                                 tag=f"v_sb{b}_{rr}", bufs=1)
                    if rr % 2 == 0:
                        nc.vector.tensor_copy(t[:], ps[:])
                    else:
                        nc.scalar.activation(
                            t[:], ps[:], mybir.ActivationFunctionType.Copy)
                    v_sb[(b, rr)] = t

            # ---------------- phase 2: attention + wo --------------------
            def attend_batch(b):
                # largest window first: deepens the pipeline early and
                # leaves the shortest window for the drain tail
                for w in (3, 2, 1, 0):
                    j = NW * b + w
                    qn_sb = []
                    t = ph2.tile([128, R], MM_DT, name="qn_sb0",
                                 tag="qn0", bufs=2)
                    nc.sync.dma_start(
                        out=t[:],
                        in_=qa2a_out[j * 384:j * 384 + 128, :])
                    qn_sb.append(t)
                    t = ph2.tile([128, R], MM_DT, name="qn_sb1",
                                 tag="qn1", bufs=2)
                    nc.sync.dma_start(
                        out=t[:],
                        in_=qa2a_out[j * 384 + 256:j * 384 + 384, :])
                    qn_sb.append(t)
                    # both heads' rope'd q in one tile: rows
                    # [y0h0; y1h0; y0h1; y1h1]
                    qpe2 = ph2.tile([128, R], MM_DT, name="qpe2",
                                    tag="qpe2", bufs=2)
                    for hh in range(HC):
                        nc.sync.dma_start(
                            out=qpe2[hh * 64:hh * 64 + 32, :],
                            in_=qa2a_out[j * 384 + 128 + hh * 32:
                                         j * 384 + 128 + (hh + 1) * 32, :])
                        nc.sync.dma_start(
                            out=qpe2[hh * 64 + 32:hh * 64 + 64, :],
                            in_=qa2a_out[j * 384 + 192 + hh * 32:
                                         j * 384 + 192 + (hh + 1) * 32, :])

                    nt = 4 * w + 4          # kv tiles in this window
                    ps_sum = ps_sm.tile([64, R], F32, name="ps_sum",
                                        tag="sum")
                    psO = [ps_o.tile([128, R], F32, name=f"psO{hh}",
                                     tag=f"o{hh}") for hh in range(HC)]
                    for t_i in range(nt):
                        d = t_i - 4 * w
                        # diagonal tiles: q columns < 128*d are fully
                        # masked, so narrow every op to the live columns
                        q0 = 128 * d if d > 0 else 0
                        N = R - q0
                        ps_s = [ps_mm.tile([128, R], F32, name="ps_s",
                                           tag="mm") for _ in range(HC)]
                        # K=64 rope scores first (start the psum groups):
                        # both have identical deps (window q tiles), so
                        # they issue back-to-back and row-pack into one
                        # matmul slot (tile_position (0,0)/(64,0)).
                        for hh in range(HC):
                            nc.tensor.matmul(
                                ps_s[hh][:, 0:N],
                                kpe_dup[b][hh * 64:(hh + 1) * 64,
                                           t_i * 128:(t_i + 1) * 128],
                                qpe2[hh * 64:(hh + 1) * 64, q0:R],
                                start=True, stop=False)
                        for hh in range(HC):
                            nc.tensor.matmul(
                                ps_s[hh][:, 0:N],
                                kT[(b, hh)][:, t_i * 128:(t_i + 1) * 128],
                                qn_sb[hh][:, q0:R], start=False, stop=True)
                        ats = [None, None]
                        for hh in range(HC):   # match score completion order
                            at = ph2.tile([128, R], MM_DT, name="attnT",
                                          tag="attnT", bufs=8)
                            nc.scalar.activation(
                                at[0:128, 0:N], ps_s[hh][:, 0:N],
                                mybir.ActivationFunctionType.Exp)
                            if d >= 0:
                                # zero strictly-above-diagonal entries
                                # (in narrowed coords: keep where f' >= p)
                                nc.gpsimd.affine_select(
                                    out=at[0:128, 0:N], in_=at[0:128, 0:N],
                                    compare_op=mybir.AluOpType.is_ge,
                                    fill=0.0, base=0,
                                    pattern=[[1, N]], channel_multiplier=-1)
                            ats[hh] = at
                        # softmax denominators, col-packed across the
                        # heads; at1 is ready last, so sum1 leads and the
                        # pair issues together.
                        for hh in (1, 0):
                            nc.tensor.matmul(
                                ps_sum[32 * hh:32 * hh + 1, q0:R],
                                ones_col[:], ats[hh][0:128, 0:N],
                                start=(t_i == 0), stop=(t_i == nt - 1))
                        for hh in (1, 0):
                            nc.tensor.matmul(
                                psO[hh][:, q0:R],
                                v_sb[(b, t_i)][:, hh * VD:(hh + 1) * VD],
                                ats[hh][0:128, 0:N], start=(t_i == 0),
                                stop=(t_i == nt - 1))
                    # normalize per head: oTn = psO * (1/sums) broadcast
                    oTn = []
                    for hh in range(HC):
                        sums = workp.tile([1, R], F32, name="sums",
                                          tag="rs_small", bufs=2)
                        nc.scalar.activation(
                            sums[:], ps_sum[32 * hh:32 * hh + 1, :],
                            mybir.ActivationFunctionType.Copy)
                        ri = workp.tile([1, R], F32, name="ri_at",
                                        tag="ri_small", bufs=2)
                        nc.vector.reciprocal_approx_fast(ri[:], sums[:])
                        rib = workp.tile([1, R], MM_DT, name="ri_at_b",
                                         tag="rib_small", bufs=2)
                        nc.vector.tensor_copy(rib[:], ri[:])
                        bcp = ps_wo.tile([128, R], F32, name="bc_at",
                                         tag="wo")
                        nc.tensor.matmul(bcp[:], ones_row[:], rib[:],
                                         start=True, stop=True)
                        bcs = ph2.tile([128, R], MM_DT, name="bc_at_sb",
                                       tag="bc_at", bufs=2)
                        nc.scalar.activation(
                            bcs[:], bcp[:],
                            mybir.ActivationFunctionType.Copy)
                        ot = ph2.tile([128, R], MM_DT, name="oTn",
                                      tag=f"oTn{hh}", bufs=2)
                        nc.vector.tensor_mul(ot[:], psO[hh][:], bcs[:])
                        oTn.append(ot)
                    # wo partials: both heads accumulate into one psum
                    for rs in range(4):
                        ob = ph2.tile([128, DIM], F16, name="ob", tag="ob",
                                      bufs=2)
                        for cp in range(4):
                            pw = ps_wo.tile([128, 512], F32, name="ps_wo",
                                            tag="wo")
                            nc.tensor.matmul(
                                pw[:],
                                oTn[0][:, rs * 128:(rs + 1) * 128],
                                wo_sb[0][:, cp * 512:(cp + 1) * 512],
                                start=True, stop=False)
                            nc.tensor.matmul(
                                pw[:],
                                oTn[1][:, rs * 128:(rs + 1) * 128],
                                wo_sb[1][:, cp * 512:(cp + 1) * 512],
                                start=False, stop=True)
                            # alternate drain engine so neither ACT nor
                            # DVE becomes the serial bottleneck
                            if cp % 2 == 0:
                                nc.vector.tensor_copy(
                                    ob[:, cp * 512:(cp + 1) * 512], pw[:])
                            else:
                                nc.scalar.activation(
                                    ob[:, cp * 512:(cp + 1) * 512], pw[:],
                                    mybir.ActivationFunctionType.Copy)
                        row0 = b * S + w * 512 + rs * 128
                        nc.sync.dma_start(out=out[row0:row0 + 128, :],
                                          in_=ob[:])

            expand_batch(0)
            attend_batch(0)
            expand_batch(1)
            attend_batch(1)
    nc.compile()
    return nc


def _get_nc():
    if "nc" not in _compiled:
        _compiled["nc"] = _build_nc()
    return _compiled["nc"]


# ---- host-side preparation ----------------------------------------------

def _pe_perm():
    """Permutation of a head's 64 rope dims: pair i -> (i, i+32)."""
    p = np.empty(ROPE, dtype=np.int64)
    for i in range(ROPE // 2):
        p[i] = 2 * i
        p[i + 32] = 2 * i + 1
    return p


def _prep_inputs(x, freqs_cos, freqs_sin,
                 wq_a_w, q_norm_w, wq_b_w,
                 wkv_a_w, kv_norm_w, wkv_b_w, wo_w):
    f32 = np.float32
    c = np.ascontiguousarray
    rows = np.asarray(x, f32).reshape(ROWS, DIM)
    pe = _pe_perm()

    wqaT = np.asarray(wq_a_w, f32).T                         # (DIM, QL)
    # partition-major packing: [p, k*QL + c] = wqaT[k*128+p, c]
    wqaP = c(wqaT.reshape(16, 128, QL).transpose(1, 0, 2)
             .reshape(128, 16 * QL))

    wkva = np.asarray(wkv_a_w, f32).copy()                   # (576, DIM)
    wkva[KVL:] = wkva[KVL + pe]
    KVD = KVL + ROPE
    wkvaP = c(wkva.T.reshape(16, 128, KVD).transpose(1, 0, 2)
              .reshape(128, 16 * KVD))

    wqb = np.asarray(wq_b_w, f32) * np.asarray(q_norm_w, f32)[None, :] * SCALE
    idx = []
    for g in range(NCORE):
        # shard col order: [nope h_even | x0 hE, x0 hO, x1 hE, x1 hO | nope h_odd]
        idx.extend(range(2 * g * QKD, 2 * g * QKD + NOPE))
        for hh in (2 * g, 2 * g + 1):      # x0 components (pair i, comp 0)
            idx.extend((hh * QKD + NOPE + 2 * np.arange(32)).tolist())
        for hh in (2 * g, 2 * g + 1):      # x1 components (pair i, comp 1)
            idx.extend((hh * QKD + NOPE + 2 * np.arange(32) + 1).tolist())
        idx.extend(range((2 * g + 1) * QKD, (2 * g + 1) * QKD + NOPE))
    wqbT = wqb[np.asarray(idx)].T                            # (QL, 3072)
    # [p, g*4608 + kq*1536 + kr*384 + c] = wqbT[(4kq+kr)*128+p, g*384+c]
    wqbP = c(wqbT.reshape(3, 4, 128, NCORE, 384)
             .transpose(2, 3, 0, 1, 4).reshape(128, NCORE * 12 * 384))

    wkvb = np.asarray(wkv_b_w, f32) * np.asarray(kv_norm_w, f32)[None, :]

    cosf = np.asarray(freqs_cos, f32)
    sinf = np.asarray(freqs_sin, f32)

    in_maps = []
    for core in range(NCORE):
        r0 = core * R
        pos0 = r0 % S
        h0, h1 = 2 * core, 2 * core + 1
        k_rows = np.concatenate([wkvb[h0 * 256:h0 * 256 + NOPE],
                                 wkvb[h1 * 256:h1 * 256 + NOPE]])
        v_rows = np.concatenate([wkvb[h0 * 256 + NOPE:h0 * 256 + 256],
                                 wkvb[h1 * 256 + NOPE:h1 * 256 + 256]])
        ct = cosf[pos0:pos0 + R].T                            # (32, R)
        st = sinf[pos0:pos0 + R].T
        xsl = rows[r0:r0 + R]                                 # (R, DIM)
        m = {
            "xP": c(xsl.reshape(R, 16, 128).transpose(2, 1, 0)
                    .reshape(128, 16 * R)),
            "wqaP": wqaP,
            "wkvaP": wkvaP,
            "wqbP": wqbP,
            "wkbT": c(k_rows.T),
            "wvbT": c(v_rows.T),
            "woT": c(wo_w[:, core * 256:core * 256 + 256].T.astype(f32)),
            "cosA": c(np.concatenate([ct, ct])),
            "sinApm": c(np.concatenate([-st, st])),
            "cosC": c(np.concatenate([ct, ct, ct, ct])),
            "sinCpm": c(np.concatenate([-st, -st, st, st])),
        }
        m = {k: v.astype(NP_MM_DT) for k, v in m.items()}
        in_maps.append(m)
    return in_maps


def kernel(x, start_pos, freqs_cos, freqs_sin, mask,
           wq_a_w, wq_a_b, q_norm_w, wq_b_w, wq_b_b,
           wkv_a_w, wkv_a_b, kv_norm_w, wkv_b_w, wkv_b_b,
           wo_w, wo_b):
    nc = _get_nc()
    in_maps = _prep_inputs(x, freqs_cos, freqs_sin,
                           wq_a_w, q_norm_w, wq_b_w,
                           wkv_a_w, kv_norm_w, wkv_b_w, wo_w)
    res = run_bass_kernel_spmd(nc, in_maps, list(range(NCORE)))
    acc = np.zeros((ROWS, DIM), np.float32)
    for core in range(NCORE):
        acc += res.results[core]["out"].astype(np.float32)
    acc += np.asarray(wo_b, np.float32)[None, :]
    return acc.reshape(B, S, DIM)


# revision 43
# speedup vs baseline: 1.0128x; 1.0128x over previous
"""MLA prefill attention kernel for 8 TRN2 NeuronCores (v2).

Sharding: phase 1 is data-parallel over rows (B*S = 4096 rows, 512/core):
x -> q_lora -> rmsnorm -> q_b (all heads) -> rope, and
x -> kv_lora -> rmsnorm / k_pe rope.  The per-row latents are then
exchanged: AllToAll moves Q^T from row-sharded to head-sharded layout,
AllGather replicates the (small) compressed kv latents.  Phase 2 is
tensor-parallel over heads (2 heads/core): expand K/V from the latents,
causal flash-style attention in score-transposed layout, then each core
computes a partial x @ wo^T for its heads' slice; the host sums the 8
fp16 partials.

Optimizations vs the original baseline (811us -> ~519us):
 - rope projections emit one fused [x0;x1] psum tile; the half-swap for
   the rotation runs as a SBUF->SBUF DMA, and the rotation itself is 3
   full-width DVE ops against [c;c]/[-s;s] tables (halves the pe matmul
   count in 1a/1c).
 - 1c computes nE/pe/nO in a single pass per shard (3 accumulating
   psum tiles) instead of two passes; the q-rmsnorm scale commutes
   through q_b and is applied at the 1c drains.
 - attention processes both heads per kv-tile; the K=64 rope-score
   matmuls of the two heads run row-packed (tile_position (0,0)/(64,0))
   and the softmax-sum matmuls run col-packed ((0,0)/(0,32)), each pair
   costing ~one matmul slot when the pair issues adjacently.
 - causal masking of diagonal tiles moved off the PE: exp runs
   unmasked, then gpsimd affine_select zeroes the upper triangle; all
   diagonal-tile ops are narrowed to the live q columns (N=512-128d).
 - wo drain: per-head softmax normalizers (reciprocal_approx_fast) are
   broadcast via a rank-1 bf16 matmul, oT is normalized once per head,
   and both heads accumulate into a single psum group with one fp16
   drain copy; partial outputs are fp16.
 - projection operands are host-packed partition-major so DMAs move
   3-16KB contiguous runs per partition (2x descriptor bandwidth).
 - K/V expansion is traced after the AllToAll (batch 1 after batch-0
   attention) so it fills the collective wait instead of blocking 1c.
"""

import numpy as np

import concourse.bass as bass
import concourse.mybir as mybir
import concourse.tile as tile
from concourse import bacc
from concourse.bass_utils import run_bass_kernel_spmd

# ---- problem constants --------------------------------------------------
NCORE = 8
B, S, DIM = 2, 2048, 2048
H = 16
QL = 1536           # q lora rank
KVL = 512           # kv lora rank
NOPE, ROPE = 128, 64
QKD = NOPE + ROPE   # 192
VD = 128
SCALE = QKD ** -0.5
EPS = float(np.finfo(np.float32).eps)
ROWS = B * S        # 4096
R = ROWS // NCORE   # 512 rows per core
HC = H // NCORE     # 2 heads per core
NW = S // 512       # 4 query windows of 512 per batch

F32 = mybir.dt.float32
F16 = mybir.dt.float16
MM_DT = mybir.dt.bfloat16
import ml_dtypes
NP_MM_DT = ml_dtypes.bfloat16

_compiled = {}


def _build_nc():
    nc = bacc.Bacc("TRN2", target_bir_lowering=False, debug=False,
                   num_devices=NCORE)

    dram_in = lambda name, shape, dt=MM_DT: nc.dram_tensor(
        name, shape, dt, kind="ExternalInput").ap()

    # projection operands are host-packed partition-major ([128, ...]) so
    # every DMA moves multi-KB contiguous runs per partition (the naive
    # [dim, R] layouts produce 1KB descriptors at ~half DMA bandwidth)
    xP = dram_in("xP", [128, 4 * 4 * R])            # x^T, 4 chunks/tile
    wqaP = dram_in("wqaP", [128, 4 * 4 * QL])       # wq_a^T packed
    wkvaP = dram_in("wkvaP", [128, 4 * 4 * (KVL + ROPE)])
    wqbP = dram_in("wqbP", [128, 8 * 12 * 384])     # (g, kq, kr, c) packed
    wkbT = dram_in("wkbT", [KVL, HC * NOPE])        # my heads' k expand
    wvbT = dram_in("wvbT", [KVL, HC * VD])          # my heads' v expand
    woT = dram_in("woT", [HC * VD, DIM])            # my heads' wo slice^T
    cosA = dram_in("cosA", [64, R])     # [c32; c32]
    sinApm = dram_in("sinApm", [64, R])  # [-s32; s32]
    cosC = dram_in("cosC", [128, R])    # [c32 x4]
    sinCpm = dram_in("sinCpm", [128, R])  # [-s32;-s32; s32;s32]
    out = nc.dram_tensor("out", [ROWS, DIM], F16, kind="ExternalOutput").ap()

    QD = H * QKD        # 3072 rows of Q^T (permuted/grouped)
    KVD = KVL + ROPE    # 576

    from contextlib import ExitStack
    with tile.TileContext(nc) as tc, ExitStack() as stk:
        dramp = stk.enter_context(tc.tile_pool(name="dram", bufs=1,
                                               space="DRAM"))
        constp = stk.enter_context(tc.tile_pool(name="const", bufs=1))
        persist = stk.enter_context(tc.tile_pool(name="persist", bufs=1))
        workp = stk.enter_context(tc.tile_pool(name="work", bufs=3))
        # stays open through phase 2 (ph2/ps_mm open above it mid-build,
        # so a mid-build close would violate pool LIFO order)
        p1qa = stk.enter_context(tc.tile_pool(name="p1_qa", bufs=1))
        ps1ab_stk = ExitStack()
        ps1 = ps1ab_stk.enter_context(tc.tile_pool(name="ps1ab", bufs=1,
                                                   space="PSUM"))
        p1x_stk = ExitStack()
        p1x = p1x_stk.enter_context(tc.tile_pool(name="p1_x", bufs=1))
        p1kv_stk = ExitStack()
        p1kv = p1kv_stk.enter_context(tc.tile_pool(name="p1_kv", bufs=1))
        if True:

            # ---------------- constants ----------------
            ones_f32 = constp.tile([128, 1], F32, name="ones_f32",
                                   tag="ones_f32")
            nc.gpsimd.memset(ones_f32, 1.0)
            ones_row_f32 = constp.tile([1, 128], F32, name="ones_row_f32",
                                       tag="ones_row_f32")
            nc.gpsimd.memset(ones_row_f32, 1.0)
            ones_col = constp.tile([128, 1], MM_DT, name="ones_col",
                                   tag="ones_col")
            nc.vector.tensor_copy(ones_col[:], ones_f32[:])
            ones_row = constp.tile([1, 128], MM_DT, name="ones_row",
                                   tag="ones_row")
            nc.vector.tensor_copy(ones_row[:], ones_row_f32[:])
            eps1 = constp.tile([1, 1], F32, name="eps1", tag="eps1")
            nc.gpsimd.memset(eps1, EPS)
            cosA_sb = constp.tile([64, R], MM_DT, name="cosA_sb",
                                  tag="cosA_sb")
            sinA_sb = constp.tile([64, R], MM_DT, name="sinA_sb",
                                  tag="sinA_sb")
            cosC_sb = constp.tile([128, R], MM_DT, name="cosC_sb",
                                  tag="cosC_sb")
            sinC_sb = constp.tile([128, R], MM_DT, name="sinC_sb",
                                  tag="sinC_sb")
            nc.sync.dma_start(out=cosA_sb[:], in_=cosA[:])
            nc.sync.dma_start(out=sinA_sb[:], in_=sinApm[:])
            nc.sync.dma_start(out=cosC_sb[:], in_=cosC[:])
            nc.sync.dma_start(out=sinC_sb[:], in_=sinCpm[:])

            # HAM warm-up: ~5us of dummy matmuls while the input DMAs
            # stream, so phase 1a starts at the unthrottled PE clock.
            ps_warm = ps1.tile([128, R], F32, name="ps_warm", tag="bc_ps")
            for _ in range(12):
                nc.tensor.matmul(ps_warm[:], cosC_sb[:, 0:128], cosC_sb[:],
                                 start=True, stop=True)
            wsink = workp.tile([1, 1], F32, name="wsink", tag="wsink")
            nc.vector.tensor_copy(wsink[:], ps_warm[0:1, 0:1])
            warm_d = dramp.tile([1, 1], F32, name="warm_d", tag="warm_d")
            nc.sync.dma_start(out=warm_d[:], in_=wsink[:])

            # x^T resident: 4 packed tiles of 4 chunks each
            x4 = []
            for q in range(4):
                t = p1x.tile([128, 4 * R], MM_DT, name=f"x4_{q}",
                             tag=f"x4_{q}")
                nc.sync.dma_start(out=t[:],
                                  in_=xP[:, q * 4 * R:(q + 1) * 4 * R])
                x4.append(t)

            def x_chunk(k):
                return x4[k // 4][:, (k % 4) * R:(k % 4 + 1) * R]

            # collective buffers
            kvag_in = dramp.tile([KVD, R], MM_DT, name="kvag_in",
                                 tag="kvag_in")
            kvag_out = dramp.tile([NCORE * KVD, R], MM_DT, name="kvag_out",
                                  tag="kvag_out", addr_space="Shared")
            qa2a_in = dramp.tile([QD, R], MM_DT, name="qa2a_in",
                                 tag="qa2a_in")
            qa2a_out = dramp.tile([QD, R], MM_DT, name="qa2a_out",
                                  tag="qa2a_out")

            # ---------------- phase 1a: kv latents (feeds AllGather) -----
            kv_dt = []     # kvnT tiles [128, R] per kvl chunk
            ssq_kv = ps1.tile([1, R], F32, name="ssq_kv", tag="ssq_small")
            ps_pe1a = ps1.tile([64, R], F32, name="ps_pe1a", tag="pe_x")
            wkva4 = []
            for q in range(4):
                wt = p1kv.tile([128, 4 * KVD], MM_DT, name=f"wkva4_{q}",
                               tag=f"wkva4_{q}")
                nc.sync.dma_start(
                    out=wt[:], in_=wkvaP[:, q * 4 * KVD:(q + 1) * 4 * KVD])
                wkva4.append(wt)

            def wkva_sl(k, c0, c1):
                base = (k % 4) * KVD
                return wkva4[k // 4][:, base + c0:base + c1]

            for k in range(DIM // 128):
                nc.tensor.matmul(ps_pe1a[:], wkva_sl(k, KVL, KVD),
                                 x_chunk(k), start=(k == 0), stop=(k == 15))
            sq_kv = []
            for blk in range(2):
                ps_kv = [ps1.tile([128, R], F32, name=f"ps_kv{d}", tag="acc",
                                  bufs=4) for d in range(2)]
                for k in range(DIM // 128):
                    for d in range(2):
                        dd = blk * 2 + d
                        nc.tensor.matmul(ps_kv[d][:],
                                         wkva_sl(k, dd * 128, (dd + 1) * 128),
                                         x_chunk(k),
                                         start=(k == 0), stop=(k == 15))
                for d in range(2):
                    dd = blk * 2 + d
                    t = p1kv.tile([128, R], MM_DT, name=f"kvnT{dd}",
                                  tag=f"kvnT{dd}")
                    nc.scalar.activation(t[:], ps_kv[d][:],
                                         mybir.ActivationFunctionType.Copy)
                    sq = p1qa.tile([128, R], MM_DT, name="sq_kv",
                                   tag=f"sq_kv{dd}")
                    nc.vector.tensor_mul(sq[:], t[:], t[:])
                    sq_kv.append(sq)
                    kv_dt.append(t)
            # ssq matmuls issued after the projection loop so they don't
            # stall the in-order PE queue mid-stream
            for dd in range(4):
                nc.tensor.matmul(ssq_kv[:], ones_col[:], sq_kv[dd][:],
                                 start=(dd == 0), stop=(dd == 3))
            # rsqrt + broadcast along partitions via rank-1 matmul
            rs_kv = workp.tile([1, R], F32, name="rs_kv", tag="rs_small",
                               bufs=2)
            nc.scalar.activation(rs_kv[:], ssq_kv[:],
                                 mybir.ActivationFunctionType.Sqrt,
                                 bias=eps1[:], scale=1.0 / KVL)
            ri_kv = workp.tile([1, R], F32, name="ri_kv", tag="ri_small",
                               bufs=2)
            nc.vector.reciprocal_approx_fast(ri_kv[:], rs_kv[:])
            rib_kv = workp.tile([1, R], MM_DT, name="rib_kv",
                                tag="rib_small", bufs=2)
            nc.vector.tensor_copy(rib_kv[:], ri_kv[:])
            bc_ps = ps1.tile([128, R], F32, name="bc_kv", tag="bc_ps")
            nc.tensor.matmul(bc_ps[:], ones_row[:], rib_kv[:],
                             start=True, stop=True)
            bc_sb = p1qa.tile([128, R], MM_DT, name="bc_kv_sb", tag="bc",
                              bufs=2)
            nc.scalar.activation(bc_sb[:], bc_ps[:],
                                 mybir.ActivationFunctionType.Copy)
            for d in range(4):
                nc.vector.tensor_mul(kv_dt[d][:], kv_dt[d][:], bc_sb[:])
                nc.sync.dma_start(out=kvag_in[d * 128:(d + 1) * 128, :],
                                  in_=kv_dt[d][:])
            # k_pe rope: [x0;x1] -> y = x*[c;c] + swap(x)*[-s;s]
            kpe_all = p1kv.tile([64, R], MM_DT, name="kpe_all", tag="kpe_all")
            nc.scalar.activation(kpe_all[:], ps_pe1a[:],
                                 mybir.ActivationFunctionType.Copy)
            kswap = p1kv.tile([64, R], MM_DT, name="kswap", tag="kswap")
            nc.sync.dma_start(out=kswap[0:32, :], in_=kpe_all[32:64, :])
            nc.sync.dma_start(out=kswap[32:64, :], in_=kpe_all[0:32, :])
            ktmp = p1kv.tile([64, R], MM_DT, name="ktmp", tag="ktmp")
            nc.vector.tensor_mul(ktmp[:], kswap[:], sinA_sb[:])
            ky = p1kv.tile([64, R], MM_DT, name="ky", tag="ky")
            nc.vector.tensor_mul(ky[:], kpe_all[:], cosA_sb[:])
            nc.vector.tensor_add(ky[:], ky[:], ktmp[:])
            nc.sync.dma_start(out=kvag_in[KVL:KVD, :], in_=ky[:])
            nc.gpsimd.collective_compute(
                "AllGather", mybir.AluOpType.bypass,
                replica_groups=[list(range(NCORE))],
                ins=[kvag_in.opt()], outs=[kvag_out.opt()])
            p1kv_stk.close()

            # ---------------- phase 1b: q latents ------------------------
            qa_dt = []
            sq_q = []
            ssq_q = ps1.tile([1, R], F32, name="ssq_q", tag="ssq_small")
            wqa4 = []
            for q in range(4):
                wt = p1x.tile([128, 4 * QL], MM_DT, name=f"wqa4_{q}",
                              tag=f"wqa4_{q}")
                nc.sync.dma_start(
                    out=wt[:], in_=wqaP[:, q * 4 * QL:(q + 1) * 4 * QL])
                wqa4.append(wt)

            def wqa_sl(k, c0, c1):
                base = (k % 4) * QL
                return wqa4[k // 4][:, base + c0:base + c1]

            for cb in range(3):         # 512-col weight block
                for sub in range(2):    # 2 dtiles at a time
                    ps_q = [ps1.tile([128, R], F32, name=f"ps_q{d}",
                            tag="acc", bufs=4) for d in range(2)]
                    for k in range(DIM // 128):
                        for d in range(2):
                            off = cb * 512 + sub * 256 + d * 128
                            nc.tensor.matmul(ps_q[d][:],
                                             wqa_sl(k, off, off + 128),
                                             x_chunk(k),
                                             start=(k == 0), stop=(k == 15))
                    for d in range(2):
                        dt_i = cb * 4 + sub * 2 + d
                        t = p1qa.tile([128, R], MM_DT, name=f"qaT{dt_i}",
                                      tag=f"qaT{dt_i}")
                        nc.scalar.activation(
                            t[:], ps_q[d][:],
                            mybir.ActivationFunctionType.Copy)
                        sq = p1qa.tile([128, R], MM_DT, name="sq_q",
                                       tag=f"sq_q{dt_i}")
                        nc.vector.tensor_mul(sq[:], t[:], t[:])
                        sq_q.append(sq)
                        qa_dt.append(t)
            for dt_i in range(12):
                nc.tensor.matmul(ssq_q[:], ones_col[:], sq_q[dt_i][:],
                                 start=(dt_i == 0), stop=(dt_i == 11))
            rs_q = workp.tile([1, R], F32, name="rs_q", tag="rs_small",
                              bufs=2)
            nc.scalar.activation(rs_q[:], ssq_q[:],
                                 mybir.ActivationFunctionType.Sqrt,
                                 bias=eps1[:], scale=1.0 / QL)
            ri_q = workp.tile([1, R], F32, name="ri_q", tag="ri_small",
                              bufs=2)
            nc.vector.reciprocal_approx_fast(ri_q[:], rs_q[:])
            rib_q = workp.tile([1, R], MM_DT, name="rib_q",
                               tag="rib_small", bufs=2)
            nc.vector.tensor_copy(rib_q[:], ri_q[:])
            bcq_ps = ps1.tile([128, R], F32, name="bc_q", tag="bc_ps")
            nc.tensor.matmul(bcq_ps[:], ones_row[:], rib_q[:],
                             start=True, stop=True)
            # the per-row rmsnorm scale commutes through q_b (it's a
            # per-token scalar), so it's applied at the 1c drains below
            # instead of to qa_dt — removes the serial 1b->1c boundary.
            bcq_sb = p1qa.tile([128, R], MM_DT, name="bc_q_sb", tag="bc",
                               bufs=2)
            nc.scalar.activation(bcq_sb[:], bcq_ps[:],
                                 mybir.ActivationFunctionType.Copy)

            p1x_stk.close()
            ps1ab_stk.close()
            # score/expansion psum pool opens before 1c so the K/V
            # expansion matmuls can fill 1c stalls and the AllToAll wait;
            # ph2 likewise so the kvag_out loads prefetch during 1c.
            ps_mm = stk.enter_context(tc.tile_pool(name="ps_mm", bufs=3,
                                                   space="PSUM"))
            ph2 = stk.enter_context(tc.tile_pool(name="ph2", bufs=1))
            p1w_stk = ExitStack()
            p1w = p1w_stk.enter_context(tc.tile_pool(name="p1_w", bufs=1))
            ps1c_stk = ExitStack()
            ps1c = ps1c_stk.enter_context(tc.tile_pool(name="ps1c", bufs=1,
                                                       space="PSUM"))

            # ---------------- phase 1c: q_b + rope -> AllToAll ------------
            # single pass per shard: nE / pe / nO accumulate in parallel
            for g in range(NCORE):
                wt4 = []
                for kq in range(3):
                    wt = p1w.tile([128, 1536], MM_DT, name="wqb4",
                                  tag="wqb4", bufs=8)
                    nc.sync.dma_start(
                        out=wt[:],
                        in_=wqbP[:, g * 4608 + kq * 1536:
                                 g * 4608 + (kq + 1) * 1536])
                    wt4.append(wt)

                def wqb_sl(k, c0, c1):
                    base = (k % 4) * 384
                    return wt4[k // 4][:, base + c0:base + c1]

                ps_nE = ps1c.tile([128, R], F32, name="ps_nE", tag="accE",
                                  bufs=2)
                ps_pe = ps1c.tile([128, R], F32, name="ps_qpe", tag="accP",
                                  bufs=2)
                ps_nO = ps1c.tile([128, R], F32, name="ps_nO", tag="accO",
                                  bufs=1)
                for k in range(QL // 128):
                    nc.tensor.matmul(ps_nE[:], wqb_sl(k, 0, 128),
                                     qa_dt[k][:],
                                     start=(k == 0), stop=(k == 11))
                    nc.tensor.matmul(ps_pe[:], wqb_sl(k, 128, 256),
                                     qa_dt[k][:],
                                     start=(k == 0), stop=(k == 11))
                    nc.tensor.matmul(ps_nO[:], wqb_sl(k, 256, 384),
                                     qa_dt[k][:],
                                     start=(k == 0), stop=(k == 11))
                stE = p1qa.tile([128, R], MM_DT, name="qoutE", tag="qout",
                                bufs=4)
                nc.vector.tensor_mul(stE[:], ps_nE[:], bcq_sb[:])
                nc.sync.dma_start(
                    out=qa2a_in[g * 384:g * 384 + 128, :], in_=stE[:])
                stO = p1qa.tile([128, R], MM_DT, name="qoutO", tag="qout",
                                bufs=4)
                nc.vector.tensor_mul(stO[:], ps_nO[:], bcq_sb[:])
                nc.sync.dma_start(
                    out=qa2a_in[g * 384 + 256:g * 384 + 384, :], in_=stO[:])
                # rope: y = x*[c;c;c;c] + swap64(x)*[-s;-s;s;s]
                # (the rmsnorm scale rides along through the rotation)
                qpe_all = p1qa.tile([128, R], MM_DT, name="qpe_all",
                                    tag="qpe_all", bufs=2)
                nc.vector.tensor_mul(qpe_all[:], ps_pe[:], bcq_sb[:])
                qswap = p1qa.tile([128, R], MM_DT, name="qswap", tag="qswap",
                                  bufs=2)
                nc.sync.dma_start(out=qswap[0:64, :], in_=qpe_all[64:128, :])
                nc.sync.dma_start(out=qswap[64:128, :], in_=qpe_all[0:64, :])
                qtmp = p1qa.tile([128, R], MM_DT, name="qtmp", tag="qtmp",
                                 bufs=2)
                nc.vector.tensor_mul(qtmp[:], qswap[:], sinC_sb[:])
                qy = p1qa.tile([128, R], MM_DT, name="qy", tag="qy", bufs=2)
                nc.vector.tensor_mul(qy[:], qpe_all[:], cosC_sb[:])
                nc.vector.tensor_add(qy[:], qy[:], qtmp[:])
                nc.sync.dma_start(
                    out=qa2a_in[g * 384 + 128:g * 384 + 256, :], in_=qy[:])
                if g == NCORE - 1:
                    last_1c = qy
            nc.gpsimd.collective_compute(
                "AllToAll", mybir.AluOpType.bypass,
                replica_groups=[list(range(NCORE))],
                ins=[qa2a_in.opt()], outs=[qa2a_out.opt()])
            ps1c_stk.close()
            p1w_stk.close()
            ps_o = stk.enter_context(tc.tile_pool(name="ps_o", bufs=1,
                                                  space="PSUM"))
            ps_wo = stk.enter_context(tc.tile_pool(name="ps_wo", bufs=2,
                                                   space="PSUM"))
            ps_sm = stk.enter_context(tc.tile_pool(name="ps_sm", bufs=1,
                                                   space="PSUM"))

            # phase-2 weights: chained behind 1c's last output via a
            # marker-cell write, so every expansion matmul is modeled
            # (and runs) strictly after 1c — they fill the AllToAll wait
            # instead of head-of-line-blocking 1c behind the AllGather.
            marker = ph2.tile([128, 1], MM_DT, name="marker", tag="marker")
            nc.vector.tensor_copy(marker[:], last_1c[:, 0:1])
            wkb_sb = []
            wvb_sb = []
            for m in range(4):
                t = persist.tile([128, HC * NOPE], MM_DT, name=f"wkb{m}",
                                 tag=f"wkb{m}")
                nc.vector.tensor_copy(t[:, 0:1], marker[:])
                nc.sync.dma_start(out=t[:], in_=wkbT[m * 128:(m + 1) * 128, :])
                wkb_sb.append(t)
                t2 = persist.tile([128, HC * VD], MM_DT, name=f"wvb{m}",
                                  tag=f"wvb{m}")
                nc.vector.tensor_copy(t2[:, 0:1], marker[:])
                nc.sync.dma_start(out=t2[:],
                                  in_=wvbT[m * 128:(m + 1) * 128, :])
                wvb_sb.append(t2)
            wo_sb = []
            for hh in range(HC):
                t = persist.tile([128, DIM], MM_DT, name=f"wo{hh}",
                                 tag=f"wo{hh}")
                nc.sync.dma_start(out=t[:],
                                  in_=woT[hh * 128:(hh + 1) * 128, :])
                wo_sb.append(t)

            # ---------------- phase 2: expansion (per batch) -------------
            # batch 0 right after the AllToAll; batch 1 is traced after
            # batch-0 attention so its matmuls are the lowest-priority
            # filler for the AllToAll wait and attention stalls.
            kT = {}      # (b, hh) -> [128, S]
            v_sb = {}    # (b, rr) -> [128, HC*VD]
            kpe_dup = {}  # b -> [128, S] (kpe duplicated in both halves)

            def expand_batch(b):
                kvg = []
                for jj in range(4):
                    j = NW * b + jj
                    row0 = j * KVD
                    tiles_m = []
                    for m in range(4):
                        t = ph2.tile([128, R], MM_DT, name="kvg",
                                     tag=f"kvg{b}_{jj}_{m}", bufs=1)
                        nc.sync.dma_start(
                            out=t[:],
                            in_=kvag_out[row0 + m * 128:row0 + (m + 1) * 128,
                                         :])
                        tiles_m.append(t)
                    kvg.append(tiles_m)
                kp = ph2.tile([128, S], MM_DT, name="kpe_dup",
                              tag=f"kpe_dup{b}", bufs=1)
                for jj in range(4):
                    j = NW * b + jj
                    row0 = j * KVD
                    nc.sync.dma_start(
                        out=kp[0:64, jj * R:(jj + 1) * R],
                        in_=kvag_out[row0 + KVL:row0 + KVD, :])
                    nc.sync.dma_start(
                        out=kp[64:128, jj * R:(jj + 1) * R],
                        in_=kvag_out[row0 + KVL:row0 + KVD, :])
                kpe_dup[b] = kp

                for hh in range(HC):
                    t = ph2.tile([128, S], MM_DT, name=f"kT{hh}",
                                 tag=f"kT{b}_{hh}", bufs=1)
                    for jj in range(4):
                        ps = ps_mm.tile([128, R], F32, name="ps_kT",
                                        tag="mm")
                        for m in range(4):
                            nc.tensor.matmul(
                                ps[:],
                                wkb_sb[m][:, hh * NOPE:(hh + 1) * NOPE],
                                kvg[jj][m][:],
                                start=(m == 0), stop=(m == 3))
                        if jj % 2 == 0:
                            nc.vector.tensor_copy(
                                t[:, jj * R:(jj + 1) * R], ps[:])
                        else:
                            nc.scalar.activation(
                                t[:, jj * R:(jj + 1) * R], ps[:],
                                mybir.ActivationFunctionType.Copy)
                    kT[(b, hh)] = t

                for rr in range(S // 128):
                    jj, sl = rr // 4, rr % 4
                    ps = ps_mm.tile([128, HC * VD], F32, name="ps_v",
                                    tag="mm")
                    for m in range(4):
                        nc.tensor.matmul(
                            ps[:],
                            kvg[jj][m][:, sl * 128:(sl + 1) * 128],
                            wvb_sb[m][:],
                            start=(m == 0), stop=(m == 3))
                    t = ph2.tile([128, HC * VD], MM_DT, name="v_sb",
                                 tag=f"v_sb{b}_{rr}", bufs=1)
                    if rr % 2 == 0:
                        nc.vector.tensor_copy(t[:], ps[:])
                    else:
                        nc.scalar.activation(
                            t[:], ps[:], mybir.ActivationFunctionType.Copy)
                    v_sb[(b, rr)] = t

            # ---------------- phase 2: attention + wo --------------------
            def attend_batch(b):
                # largest window first: deepens the pipeline early and
                # leaves the shortest window for the drain tail
                for w in (3, 2, 1, 0):
                    j = NW * b + w
                    qn_sb = []
                    t = ph2.tile([128, R], MM_DT, name="qn_sb0",
                                 tag="qn0", bufs=2)
                    nc.sync.dma_start(
                        out=t[:],
                        in_=qa2a_out[j * 384:j * 384 + 128, :])
                    qn_sb.append(t)
                    t = ph2.tile([128, R], MM_DT, name="qn_sb1",
                                 tag="qn1", bufs=2)
                    nc.sync.dma_start(
                        out=t[:],
                        in_=qa2a_out[j * 384 + 256:j * 384 + 384, :])
                    qn_sb.append(t)
                    # both heads' rope'd q in one tile: rows
                    # [y0h0; y1h0; y0h1; y1h1]
                    qpe2 = ph2.tile([128, R], MM_DT, name="qpe2",
                                    tag="qpe2", bufs=2)
                    for hh in range(HC):
                        nc.sync.dma_start(
                            out=qpe2[hh * 64:hh * 64 + 32, :],
                            in_=qa2a_out[j * 384 + 128 + hh * 32:
                                         j * 384 + 128 + (hh + 1) * 32, :])
                        nc.sync.dma_start(
                            out=qpe2[hh * 64 + 32:hh * 64 + 64, :],
                            in_=qa2a_out[j * 384 + 192 + hh * 32:
                                         j * 384 + 192 + (hh + 1) * 32, :])

                    nt = 4 * w + 4          # kv tiles in this window
                    ps_sum = ps_sm.tile([64, R], F32, name="ps_sum",
                                        tag="sum")
                    psO = [ps_o.tile([128, R], F32, name=f"psO{hh}",
                                     tag=f"o{hh}") for hh in range(HC)]
                    for t_i in range(nt):
                        d = t_i - 4 * w
                        # diagonal tiles: q columns < 128*d are fully
                        # masked, so narrow every op to the live columns
                        q0 = 128 * d if d > 0 else 0
                        N = R - q0
                        ps_s = [ps_mm.tile([128, R], F32, name="ps_s",
                                           tag="mm") for _ in range(HC)]
                        # K=64 rope scores first (start the psum groups):
                        # both have identical deps (window q tiles), so
                        # they issue back-to-back and row-pack into one
                        # matmul slot (tile_position (0,0)/(64,0)).
                        for hh in range(HC):
                            nc.tensor.matmul(
                                ps_s[hh][:, 0:N],
                                kpe_dup[b][hh * 64:(hh + 1) * 64,
                                           t_i * 128:(t_i + 1) * 128],
                                qpe2[hh * 64:(hh + 1) * 64, q0:R],
                                start=True, stop=False)
                        for hh in range(HC):
                            nc.tensor.matmul(
                                ps_s[hh][:, 0:N],
                                kT[(b, hh)][:, t_i * 128:(t_i + 1) * 128],
                                qn_sb[hh][:, q0:R], start=False, stop=True)
                        ats = [None, None]
                        for hh in range(HC):   # match score completion order
                            at = ph2.tile([128, R], MM_DT, name="attnT",
                                          tag="attnT", bufs=8)
                            nc.scalar.activation(
                                at[0:128, 0:N], ps_s[hh][:, 0:N],
                                mybir.ActivationFunctionType.Exp)
                            if d >= 0:
                                # zero strictly-above-diagonal entries
                                # (in narrowed coords: keep where f' >= p)
                                nc.gpsimd.affine_select(
                                    out=at[0:128, 0:N], in_=at[0:128, 0:N],
                                    compare_op=mybir.AluOpType.is_ge,
                                    fill=0.0, base=0,
                                    pattern=[[1, N]], channel_multiplier=-1)
                            ats[hh] = at
                        # softmax denominators, col-packed across the
                        # heads; at1 is ready last, so sum1 leads and the
                        # pair issues together.
                        for hh in (1, 0):
                            nc.tensor.matmul(
                                ps_sum[32 * hh:32 * hh + 1, q0:R],
                                ones_col[:], ats[hh][0:128, 0:N],
                                start=(t_i == 0), stop=(t_i == nt - 1))
                        for hh in (1, 0):
                            nc.tensor.matmul(
                                psO[hh][:, q0:R],
                                v_sb[(b, t_i)][:, hh * VD:(hh + 1) * VD],
                                ats[hh][0:128, 0:N], start=(t_i == 0),
                                stop=(t_i == nt - 1))
                    # normalize per head: oTn = psO * (1/sums) broadcast
                    oTn = []
                    for hh in range(HC):
                        sums = workp.tile([1, R], F32, name="sums",
                                          tag="rs_small", bufs=2)
                        nc.scalar.activation(
                            sums[:], ps_sum[32 * hh:32 * hh + 1, :],
                            mybir.ActivationFunctionType.Copy)
                        ri = workp.tile([1, R], F32, name="ri_at",
                                        tag="ri_small", bufs=2)
                        nc.vector.reciprocal_approx_fast(ri[:], sums[:])
                        rib = workp.tile([1, R], MM_DT, name="ri_at_b",
                                         tag="rib_small", bufs=2)
                        nc.vector.tensor_copy(rib[:], ri[:])
                        bcp = ps_wo.tile([128, R], F32, name="bc_at",
                                         tag="wo")
                        nc.tensor.matmul(bcp[:], ones_row[:], rib[:],
                                         start=True, stop=True)
                        bcs = ph2.tile([128, R], MM_DT, name="bc_at_sb",
                                       tag="bc_at", bufs=2)
                        nc.scalar.activation(
                            bcs[:], bcp[:],
                            mybir.ActivationFunctionType.Copy)
                        ot = ph2.tile([128, R], MM_DT, name="oTn",
                                      tag=f"oTn{hh}", bufs=2)
                        nc.vector.tensor_mul(ot[:], psO[hh][:], bcs[:])
                        oTn.append(ot)
                    # wo partials: both heads accumulate into one psum
                    for rs in range(4):
                        ob = ph2.tile([128, DIM], F16, name="ob", tag="ob",
                                      bufs=2)
                        for cp in range(4):
                            pw = ps_wo.tile([128, 512], F32, name="ps_wo",
                                            tag="wo")
                            nc.tensor.matmul(
                                pw[:],
                                oTn[0][:, rs * 128:(rs + 1) * 128],
                                wo_sb[0][:, cp * 512:(cp + 1) * 512],
                                start=True, stop=False)
                            nc.tensor.matmul(
                                pw[:],
                                oTn[1][:, rs * 128:(rs + 1) * 128],
                                wo_sb[1][:, cp * 512:(cp + 1) * 512],
                                start=False, stop=True)
                            # alternate drain engine so neither ACT nor
                            # DVE becomes the serial bottleneck
                            if cp % 2 == 0:
                                nc.vector.tensor_copy(
                                    ob[:, cp * 512:(cp + 1) * 512], pw[:])
                            else:
                                nc.scalar.activation(
                                    ob[:, cp * 512:(cp + 1) * 512], pw[:],
                                    mybir.ActivationFunctionType.Copy)
                        row0 = b * S + w * 512 + rs * 128
                        nc.sync.dma_start(out=out[row0:row0 + 128, :],
                                          in_=ob[:])

            expand_batch(0)
            attend_batch(0)
            expand_batch(1)
            attend_batch(1)
    nc.compile()
    return nc


def _get_nc():
    if "nc" not in _compiled:
        _compiled["nc"] = _build_nc()
    return _compiled["nc"]


# ---- host-side preparation ----------------------------------------------

def _pe_perm():
    """Permutation of a head's 64 rope dims: pair i -> (i, i+32)."""
    p = np.empty(ROPE, dtype=np.int64)
    for i in range(ROPE // 2):
        p[i] = 2 * i
        p[i + 32] = 2 * i + 1
    return p


def _prep_inputs(x, freqs_cos, freqs_sin,
                 wq_a_w, q_norm_w, wq_b_w,
                 wkv_a_w, kv_norm_w, wkv_b_w, wo_w):
    f32 = np.float32
    c = np.ascontiguousarray
    rows = np.asarray(x, f32).reshape(ROWS, DIM)
    pe = _pe_perm()

    wqaT = np.asarray(wq_a_w, f32).T                         # (DIM, QL)
    # partition-major packing: [p, k*QL + c] = wqaT[k*128+p, c]
    wqaP = c(wqaT.reshape(16, 128, QL).transpose(1, 0, 2)
             .reshape(128, 16 * QL))

    wkva = np.asarray(wkv_a_w, f32).copy()                   # (576, DIM)
    wkva[KVL:] = wkva[KVL + pe]
    KVD = KVL + ROPE
    wkvaP = c(wkva.T.reshape(16, 128, KVD).transpose(1, 0, 2)
              .reshape(128, 16 * KVD))

    wqb = np.asarray(wq_b_w, f32) * np.asarray(q_norm_w, f32)[None, :] * SCALE
    idx = []
    for g in range(NCORE):
        # shard col order: [nope h_even | x0 hE, x0 hO, x1 hE, x1 hO | nope h_odd]
        idx.extend(range(2 * g * QKD, 2 * g * QKD + NOPE))
        for hh in (2 * g, 2 * g + 1):      # x0 components (pair i, comp 0)
            idx.extend((hh * QKD + NOPE + 2 * np.arange(32)).tolist())
        for hh in (2 * g, 2 * g + 1):      # x1 components (pair i, comp 1)
            idx.extend((hh * QKD + NOPE + 2 * np.arange(32) + 1).tolist())
        idx.extend(range((2 * g + 1) * QKD, (2 * g + 1) * QKD + NOPE))
    wqbT = wqb[np.asarray(idx)].T                            # (QL, 3072)
    # [p, g*4608 + kq*1536 + kr*384 + c] = wqbT[(4kq+kr)*128+p, g*384+c]
    wqbP = c(wqbT.reshape(3, 4, 128, NCORE, 384)
             .transpose(2, 3, 0, 1, 4).reshape(128, NCORE * 12 * 384))

    wkvb = np.asarray(wkv_b_w, f32) * np.asarray(kv_norm_w, f32)[None, :]

    cosf = np.asarray(freqs_cos, f32)
    sinf = np.asarray(freqs_sin, f32)

    in_maps = []
    for core in range(NCORE):
        r0 = core * R
        pos0 = r0 % S
        h0, h1 = 2 * core, 2 * core + 1
        k_rows = np.concatenate([wkvb[h0 * 256:h0 * 256 + NOPE],
                                 wkvb[h1 * 256:h1 * 256 + NOPE]])
        v_rows = np.concatenate([wkvb[h0 * 256 + NOPE:h0 * 256 + 256],
                                 wkvb[h1 * 256 + NOPE:h1 * 256 + 256]])
        ct = cosf[pos0:pos0 + R].T                            # (32, R)
        st = sinf[pos0:pos0 + R].T
        xsl = rows[r0:r0 + R]                                 # (R, DIM)
        m = {
            "xP": c(xsl.reshape(R, 16, 128).transpose(2, 1, 0)
                    .reshape(128, 16 * R)),
            "wqaP": wqaP,
            "wkvaP": wkvaP,
            "wqbP": wqbP,
            "wkbT": c(k_rows.T),
            "wvbT": c(v_rows.T),
            "woT": c(wo_w[:, core * 256:core * 256 + 256].T.astype(f32)),
            "cosA": c(np.concatenate([ct, ct])),
            "sinApm": c(np.concatenate([-st, st])),
            "cosC": c(np.concatenate([ct, ct, ct, ct])),
            "sinCpm": c(np.concatenate([-st, -st, st, st])),
        }
        m = {k: v.astype(NP_MM_DT) for k, v in m.items()}
        in_maps.append(m)
    return in_maps


def kernel(x, start_pos, freqs_cos, freqs_sin, mask,
           wq_a_w, wq_a_b, q_norm_w, wq_b_w, wq_b_b,
           wkv_a_w, wkv_a_b, kv_norm_w, wkv_b_w, wkv_b_b,
           wo_w, wo_b):
    nc = _get_nc()
    in_maps = _prep_inputs(x, freqs_cos, freqs_sin,
                           wq_a_w, q_norm_w, wq_b_w,
                           wkv_a_w, kv_norm_w, wkv_b_w, wo_w)
    res = run_bass_kernel_spmd(nc, in_maps, list(range(NCORE)))
    acc = np.zeros((ROWS, DIM), np.float32)
    for core in range(NCORE):
        acc += res.results[core]["out"].astype(np.float32)
    acc += np.asarray(wo_b, np.float32)[None, :]
    return acc.reshape(B, S, DIM)
